# revision 33
# baseline (speedup 1.0000x reference)
"""Trainium2 Bass kernel for nn_DiscriminatorBlock_38878043963811.

Strategy
--------
Data-parallel over batch: 16 images -> 8 cores x 2 images. No collectives.

Algebraic restructuring (exact up to bf16, host-side folds):
  sin(img) precomputed on host, zero-padded, in a flat 130-pitch layout; loaded
  as "s27": 27 partitions = all (dh, dw, rgb) shifted replicas, each one
  contiguous DMA.
  The ENTIRE linear path (fromrgb 1x1 + vertical & horizontal depthwise +
  low-rank residual + point 1x1) is one 3x3x3->512 conv = a single K=27
  matmul z_pre = M27 @ s27, with M27 host-composed. (clamp provably inactive.)
  z = prelu(z_pre)*sqrt(2)/64 via one ACT op (gain + both FIR norms folded).
  FIR-v (taps [1,3,3,1], stride 2) runs on PE as scaled-identity accumulating
  matmuls into PSUM; FIR-h runs on DVE over w-deinterleaved bf16 rows so all
  stride-2 reads are unit-stride; final fp32 conversion on ACT.
"""

import sys

sys.path.insert(0, "/opt/trn_rl_repo")

import numpy as np
import ml_dtypes

import concourse.bass as bass
import concourse.bacc as bacc
import concourse.tile as tile
from concourse import mybir
from concourse.bass_utils import run_bass_kernel_spmd

f32 = mybir.dt.float32
bf16 = mybir.dt.bfloat16
AF = mybir.ActivationFunctionType
ALU = mybir.AluOpType

# ---- problem constants (hardcoded; kernel.py must be self-contained) ----
B, IMG_C, IN_C, OUT_C, S = 16, 3, 256, 512, 128
HIDDEN = IN_C
KGEN_IN = 32
KSIZE = 3
N_CORES = 8
B_LOC = B // N_CORES            # 2 images per core
HC = 32                         # z-rows per chunk
NCHUNK = S // HC                # 4 chunks per image
GDW = np.float32(1.0 / np.sqrt(KSIZE))
ACT_SCALE = float(np.sqrt(2.0) / 64.0)

_CACHE = {}


def _sample_weight_np(grid, coeff, gauss_sigma, gauss_x, low_filter):
    """numpy port of reference._sample_weight (fp32)."""
    basis = np.sin(grid * np.float32(2.0 * np.pi)) * np.float32(np.exp(-0.5))
    w = coeff @ basis / np.float32(np.sqrt(HIDDEN))
    w = w - w.mean(dtype=np.float32)
    w = w * (1.0 / np.sqrt(np.mean(w * w, axis=0, keepdims=True, dtype=np.float32) + 1e-8))
    gs = 1.0 + gauss_sigma ** 2 / 5.0
    w = (w * np.exp(-(gauss_x ** 2) / (2.0 * gs))).astype(np.float32)
    nt = low_filter.shape[0]
    T = w.shape[1] - nt + 1
    out = np.empty((w.shape[0], T), np.float32)
    for t in range(T):
        out[:, t] = (w[:, t : t + nt] * low_filter[None, :]).sum(axis=1)
    return out[:, ::2]


def _build_program():
    nc = bacc.Bacc(None, target_bir_lowering=False)
    s_d = nc.declare_dram_parameter("s", [B_LOC, IMG_C, 130 * 130 + 262], bf16, isOutput=False)
    m27_d = nc.declare_dram_parameter("m27", [27, OUT_C], bf16, isOutput=False)
    id_d = nc.declare_dram_parameter("ident", [128, 256], bf16, isOutput=False)
    pb_d = nc.declare_dram_parameter("pb", [OUT_C, 1], f32, isOutput=False)
    out_d = nc.declare_dram_parameter("out", [B_LOC, OUT_C, S // 2, S // 2], f32, isOutput=True)

    SROWS = HC + 2  # 34 rows held per chunk (1-row halo each side)

    with tile.TileContext(nc) as tc:
        with (
            tc.tile_pool(name="const", bufs=1) as cpool,
            tc.tile_pool(name="spool", bufs=3) as spool,
            tc.tile_pool(name="zpool", bufs=3) as zpool,
            tc.tile_pool(name="fir", bufs=3) as fpool,
            tc.tile_pool(name="o1pool", bufs=4) as o1pool,
            tc.tile_pool(name="o2pool", bufs=3) as o2pool,
            tc.tile_pool(name="zpsum", bufs=2, space="PSUM") as zpsum,
            tc.tile_pool(name="firps", bufs=2, space="PSUM") as firps,
        ):
            # ---- load constants ----
            m27t = cpool.tile([27, OUT_C], bf16)
            nc.sync.dma_start(m27t[:], m27_d[:])
            idt = cpool.tile([128, 256], bf16)
            nc.sync.dma_start(idt[:], id_d[:])
            pbt = [cpool.tile([128, 1], f32, tag=f"pb{i}", name=f"pb{i}") for i in range(4)]
            for i in range(4):
                nc.sync.dma_start(pbt[i][:], pb_d[i * 128 : (i + 1) * 128, :])
            zrow = cpool.tile([128, 128], bf16)
            nc.vector.memset(zrow[:], 0.0)

            R = HC // 2

            def fir_block(b, bk, ztiles_bk, ztiles_nxt):
                """FIR-v + FIR-h + store for out rows [R*bk, R*bk+R)."""
                for mt in range(4):
                    zt = ztiles_bk[mt]
                    z3 = zt[:].rearrange("p (r w) -> p r w", w=128)
                    zv = zt[:].rearrange("p (r2 two w) -> p r2 two w", two=2, w=128)
                    if ztiles_nxt is not None:
                        nxt_row1 = ztiles_nxt[mt][:].rearrange("p (r w) -> p r w", w=128)[:, 1:2, :]
                    else:
                        nxt_row1 = zrow[:].rearrange("p (r w) -> p r w", w=128)[:, 0:1, :]
                    # out1 = z[2ho-1] + 3 z[2ho] + 3 z[2ho+1] + z[2ho+2] on PE via
                    # scaled-identity accumulating matmuls (N=512 chunks)
                    o1t = o1pool.tile([128, R * 128], bf16, tag="o1", name=f"o1_{b}_{bk}_{mt}")
                    for half in range(2):
                        fp = firps.tile([128, 1024], f32, tag="fp", name=f"fp{b}_{bk}_{mt}_{half}")
                        for seg in range(2):
                            i0 = 8 * half + 4 * seg
                            o = fp[:, seg * 512 : seg * 512 + 512]
                            nc.tensor.matmul(o, idt[:, 0:128], zv[:, i0 : i0 + 4, 0, :],
                                             start=True, stop=False)
                            nc.tensor.matmul(o, idt[:, 128:256], zv[:, i0 : i0 + 4, 1, :],
                                             start=False, stop=False)
                            nc.tensor.matmul(o, idt[:, 128:256], zv[:, i0 + 1 : i0 + 5, 0, :],
                                             start=False, stop=False)
                            if i0 < 12:
                                nc.tensor.matmul(o, idt[:, 0:128], zv[:, i0 + 1 : i0 + 5, 1, :],
                                                 start=False, stop=True)
                            else:  # last row's z[2ho+2] lives in the next chunk
                                nc.tensor.matmul(o[:, 0:384], idt[:, 0:128], zv[:, 13:16, 1, :],
                                                 start=False, stop=True)
                                nc.tensor.matmul(o[:, 384:512], idt[:, 0:128], nxt_row1,
                                                 start=False, stop=True)
                        dst = o1t[:, half * 1024 : half * 1024 + 1024]
                        if mt == 0:
                            nc.scalar.activation(dst, fp[:], AF.Copy, bias=0.0, scale=1.0)
                        else:
                            nc.vector.tensor_copy(dst, fp[:])
                    # ---- FIR-h on deinterleaved rows [64 even | 64 odd] ----
                    o3 = o1t[:].rearrange("p (r w) -> p r w", w=128)
                    qt = fpool.tile([128, R * 64], bf16, tag="fq", name=f"fq{b}_{bk}_{mt}")
                    q3 = qt[:].rearrange("p (r w) -> p r w", w=64)
                    # q[j] = 3*odd[j] + even[j+1]  (j=0..62), q[63] = 3*odd[63]
                    nc.vector.scalar_tensor_tensor(q3[:, :, 0:63], o3[:, :, 64:127], 3.0, o3[:, :, 1:64], ALU.mult, ALU.add)
                    nc.vector.tensor_scalar_mul(q3[:, :, 63:64], o3[:, :, 127:128], 3.0)
                    o2t = o2pool.tile([128, R * 64], bf16, tag="o2", name=f"o2_{b}_{bk}_{mt}")
                    o23 = o2t[:].rearrange("p (r w) -> p r w", w=64)
                    # out2 = 3*even[j] + q[j] (+ odd[j-1] for j>=1)   (bf16, 2x mode)
                    nc.vector.scalar_tensor_tensor(o23[:, :, :], o3[:, :, 0:64], 3.0, q3[:, :, :], ALU.mult, ALU.add)
                    nc.vector.tensor_add(o23[:, :, 1:64], o23[:, :, 1:64], o3[:, :, 64:127])
                    o2f = o2pool.tile([128, R * 64], f32, tag="o2f", name=f"o2f_{b}_{bk}_{mt}")
                    if mt < 2:
                        nc.scalar.activation(o2f[:], o2t[:], AF.Copy, bias=0.0, scale=1.0)
                    else:
                        nc.vector.tensor_copy(o2f[:], o2t[:])
                    oeng = nc.sync if mt % 2 == 0 else nc.gpsimd
                    oeng.dma_start(
                        out_d[b, mt * 128 : (mt + 1) * 128, bk * R : (bk + 1) * R, :],
                        o2f[:].rearrange("p (r w) -> p r w", w=64),
                    )

            for b in range(B_LOC):
                prev_z = None
                for j in range(NCHUNK):
                    # ---- build s9: 9 partitions (r, jshift) of zero-padded sin,
                    # each a contiguous flat copy with offset jj (pitch 130) ----
                    s9 = spool.tile([27, SROWS * 130], bf16, tag="s9", name=f"s9_{b}_{j}")
                    s93v = s9[:].rearrange("p (r w) -> p r w", w=130)
                    lo = HC * j - 1
                    start = (lo + 1) * 130
                    for d in range(3):   # partition layout p = (d*3 + jj)*3 + r
                        for jj in range(3):
                            eng = nc.sync if (d * 3 + jj) % 2 == 0 else nc.gpsimd
                            p0 = (d * 3 + jj) * 3
                            off = start + d * 130 + jj
                            eng.dma_start(
                                s9[p0 : p0 + 3, :],
                                s_d[b, :, off : off + SROWS * 130],
                            )

                    # ---- z tiles for this chunk (34 rows: row0 = halo z[32j-1]) ----
                    ztiles = [zpool.tile([128, (HC + 2) * 128], bf16, tag=f"z{mt}", name=f"z{mt}_{b}_{j}") for mt in range(4)]
                    for mt in range(4):
                        z3 = ztiles[mt][:].rearrange("p (r w) -> p r w", w=128)
                        if j == 0:
                            nc.vector.memset(z3[:, 0:1, :], 0.0)
                        else:
                            nc.vector.tensor_copy(
                                z3[:, 0:1, :],
                                prev_z[mt][:].rearrange("p (r w) -> p r w", w=128)[:, HC : HC + 1, :],
                            )

                    # ---- z matmuls (whole linear path fused) + prelu evac ----
                    for t in range(HC // 8):  # 1024-px stretches (8 image rows)
                        for mt in range(4):
                            zp = zpsum.tile([128, 1024], f32, tag="zp", name=f"zp_{b}_{j}_{t}_{mt}")
                            for nn in range(2):
                                hl = 8 * t + 4 * nn
                                nc.tensor.matmul(zp[:, nn * 512 : nn * 512 + 512],
                                                 m27t[:, mt * 128 : mt * 128 + 128],
                                                 s93v[:, hl : hl + 4, 0:128],
                                                 start=True, stop=True)
                            zpv = zp[:].rearrange("p (r w2 two) -> p r two w2", two=2, w2=64)
                            zdst = ztiles[mt][:].rearrange("p (r par w2) -> p r par w2", par=2, w2=64)
                            rows = zdst[:, 1 + 8 * t : 9 + 8 * t, :, :]
                            nc.scalar.activation(rows, zpv, AF.Prelu,
                                                 bias=pbt[mt][:, 0:1], scale=ACT_SCALE, alpha=0.2)

                    # ---- FIR for previous block (needs this chunk's z row 1) ----
                    if j > 0:
                        fir_block(b, j - 1, prev_z, ztiles)
                    prev_z = ztiles
                fir_block(b, NCHUNK - 1, prev_z, None)

    nc.compile()
    return nc


def kernel(**inputs):
    inputs = {k: np.asarray(v) for k, v in inputs.items()}
    img = inputs["img"].astype(np.float32)
    assert img.shape == (B, IMG_C, S, S)

    # ---- host-side weight generation (tiny) ----
    freqs = inputs["freqs"].astype(np.float32)
    phases = inputs["phases"].astype(np.float32)
    g = ((np.arange(KGEN_IN, dtype=np.float32) - (KGEN_IN - 1) / 2.0)
         * np.float32(2.0 / (KGEN_IN + 1)))
    gsig = np.float32(inputs["gauss_sigma"])
    gx = inputs["gauss_x"].astype(np.float32)
    lf = inputs["low_filter"].astype(np.float32)
    hz = _sample_weight_np(freqs[:, 0:1] * g[None, :] + phases[:, None],
                           inputs["hz_outdim"].astype(np.float32), gsig, gx, lf)
    vt = _sample_weight_np(freqs[:, 1:2] * g[None, :] + phases[:, None],
                           inputs["vt_outdim"].astype(np.float32), gsig, gx, lf)

    Wfr = inputs["fromrgb_w"][:, :, 0, 0].astype(np.float32) * np.float32(1.0 / np.sqrt(IMG_C))
    assert np.abs(Wfr).sum(1).max() < 250.0, "fromrgb clamp would be active"
    assert np.all(inputs["fromrgb_b"] == 0.0), "nonzero fromrgb bias unsupported"

    # k27[(d*3+jj)*3+r, c] = vt[c,d]*hz[c,jj]*GDW^2*Wfr[c,r]
    k9_np = np.zeros((27, IN_C), np.float32)
    for d in range(3):
        for r in range(3):
            for jj in range(3):
                k9_np[(d * 3 + jj) * 3 + r, :] = (
                    vt[:, d] * hz[:, jj] * GDW * GDW * Wfr[:, r]
                )
    L = inputs["lr_weight0"][:, :, 0, 0].astype(np.float32) * np.float32(1.0 / np.sqrt(IN_C))
    Pp = inputs["point_w"][:, :, 0, 0].astype(np.float32) * np.float32(1.0 / np.sqrt(IN_C))
    plw3 = (Pp @ L @ Wfr).T                      # [3, 512]
    # whole linear path: z_pre = M27 @ s27, M27 = K27 P^T + PLW27
    m27_np = k9_np @ Pp.T                        # [27, 512]
    for r in range(3):
        m27_np[12 + r] += plw3[r]
    pb_np = (inputs["point_b"].astype(np.float32) * np.float32(ACT_SCALE)).reshape(OUT_C, 1)

    spad = np.zeros((B, IMG_C, 130, 130), np.float32)
    spad[:, :, 1:129, 1:129] = np.sin(img)
    s_np = np.zeros((B, IMG_C, 130 * 130 + 262), np.float32)
    s_np[:, :, : 130 * 130] = spad.reshape(B, IMG_C, -1)
    s_np = s_np.astype(ml_dtypes.bfloat16)
    id_np = np.zeros((128, 256), np.float32)
    id_np[:, 0:128] = np.eye(128)
    id_np[:, 128:256] = 3.0 * np.eye(128)
    shared = dict(
        m27=m27_np.astype(ml_dtypes.bfloat16),
        ident=id_np.astype(ml_dtypes.bfloat16),
        pb=pb_np,
    )
    in_maps = [dict(s=np.ascontiguousarray(s_np[c * B_LOC : (c + 1) * B_LOC]), **shared)
               for c in range(N_CORES)]

    if "nc" not in _CACHE:
        _CACHE["nc"] = _build_program()
    res = run_bass_kernel_spmd(_CACHE["nc"], in_maps, list(range(N_CORES)),
                               **_CACHE.get("run_kwargs", {}))
    _CACHE["last"] = res
    out = np.concatenate([res.results[c]["out"] for c in range(N_CORES)], axis=0)
    return out.astype(np.float32)
